# revision 5
# baseline (speedup 1.0000x reference)
"""Trainium2 Bass kernel: cross-entropy with Gaussian-smoothed labels.

loss = mean over tokens of  [ Wsum(t) * logsumexp(pred_row) - sum_k w_k * pred[start+k] ]

where the smoothed one-hot reduces exactly to a 7-tap window:
  start = clip(t-3, 0, C-7), u = t-start, w_k = f(k-u)
  f(0)=1.0, f(+-m)=exp(-2^m/4) for m in 1..3, else 0.

Sharding: pure data-parallel over the batch axis, 4 batches (8192 tokens)
per core across 8 cores. Per core:
  - stream pred [8192, 722] f32 through SBUF in [128, G*722] tiles.
    Per-token sum(exp) is load-balanced between the ACT engine
    (exp with HW accumulator, one op per token column) and DVE
    (one batched Exp + reduce_sum per group) so neither engine
    outruns the DMA stream; Ln is split so the ln-table load hides
    under the stream.
  - 64 indirect DMAs gather the 7-wide windows (one offset per
    partition per op — the only offset layout the SWDGE firmware
    honors; multi-offset-per-partition reads garbage on HW even
    though CoreSim accepts it).
  - weights built once on-chip from iota/compares ([128, 64, 7]);
    the whole gather-side term is computed early, off the tail.
  - per-core partial sums [128, 1] DMA'd out; host sums 8x128 and divides.
"""
import math

import numpy as np

import concourse.bass as bass
import concourse.bacc as bacc
import concourse.tile as tile
from concourse import mybir
from concourse import bass_utils

B, T, C = 32, 2048, 722
CORES = 8
SHARD = B * T // CORES          # 8192 tokens per core
P = 128
TILES = SHARD // P              # 64
K = 7
START_MAX = C - K               # 715
DECAYS = [math.exp(-(2.0 ** d) / 4.0) for d in range(4)]

_ALU = mybir.AluOpType
_ACT = mybir.ActivationFunctionType

_NC = None


def _bcast_inner(ap, n):
    """Append a step-0 broadcast dim of length n to an AP."""
    return bass.AP(tensor=ap.tensor, offset=ap.offset, ap=[*ap.ap, [0, n]])


def _build(G=4, acc_set=(3, 6, 9, 12, 15), ln_split=48, pred_bufs=4, exp_bufs=3):
    ngroups = TILES // G
    acc_set = set(acc_set)
    nc = bacc.Bacc("TRN2", target_bir_lowering=False, debug=False,
                   enable_asserts=True, num_devices=CORES)
    pred = nc.dram_tensor("pred", [SHARD, C], mybir.dt.float32, kind="ExternalInput")
    target = nc.dram_tensor("target", [SHARD], mybir.dt.int32, kind="ExternalInput")
    out = nc.dram_tensor("partial", [P, 1], mybir.dt.float32, kind="ExternalOutput")

    pred_flat = pred.ap().rearrange("a b -> (a b)").rearrange("(n one) -> n one", one=1)
    # token index = p*TILES + jg*G + g  (each partition owns a contiguous slab)
    pred_g = pred.ap().rearrange("(p j g) c -> j p g c", p=P, g=G)

    with tile.TileContext(nc) as tc:
        with (tc.tile_pool(name="pred", bufs=pred_bufs) as pred_pool,
              tc.tile_pool(name="exp", bufs=exp_bufs) as exp_pool,
              tc.tile_pool(name="small", bufs=1) as small):
            # targets: tgt_sb[p, j] = target[j*128 + p]; scalar ring keeps
            # the sync ring exclusively for the pred stream.
            tgt_sb = small.tile([P, TILES], mybir.dt.int32)
            nc.scalar.dma_start(out=tgt_sb,
                                in_=target.ap().rearrange("(p j) -> p j", p=P))

            # flat element offsets of each token's window start
            row = small.tile([P, TILES], mybir.dt.int32)
            nc.gpsimd.iota(row, pattern=[[1, TILES]], base=0, channel_multiplier=TILES)
            start_i = small.tile([P, TILES], mybir.dt.int32)
            nc.vector.tensor_scalar(out=start_i, in0=tgt_sb, scalar1=3, scalar2=0,
                                    op0=_ALU.subtract, op1=_ALU.max)
            nc.vector.tensor_scalar_min(out=start_i, in0=start_i, scalar1=START_MAX)
            offs = small.tile([P, TILES], mybir.dt.int32)
            nc.vector.tensor_scalar_mul(out=offs, in0=row, scalar1=C)
            nc.vector.tensor_add(out=offs, in0=offs, in1=start_i)

            # windowed gathers: one indirect DMA per token-tile, one offset
            # per partition (the only HW-correct layout)
            gath = small.tile([P, TILES, K], mybir.dt.float32)
            for j in range(TILES):
                nc.gpsimd.indirect_dma_start(
                    out=gath[:, j, :],
                    out_offset=None,
                    in_=pred_flat,
                    in_offset=bass.IndirectOffsetOnAxis(ap=offs[:, j:j + 1], axis=0),
                )

            # u = t - start in f32 (0..6); diff[p,j,k] = k - u[p,j]
            tf = small.tile([P, TILES], mybir.dt.float32)
            nc.vector.tensor_copy(out=tf, in_=tgt_sb)
            sf = small.tile([P, TILES], mybir.dt.float32)
            nc.vector.tensor_copy(out=sf, in_=start_i)
            uf = small.tile([P, TILES], mybir.dt.float32)
            nc.vector.tensor_sub(out=uf, in0=tf, in1=sf)

            iok = small.tile([P, TILES, K], mybir.dt.float32)
            nc.gpsimd.iota(iok, pattern=[[0, TILES], [1, K]], base=0,
                           channel_multiplier=0, allow_small_or_imprecise_dtypes=True)
            diff = small.tile([P, TILES, K], mybir.dt.float32)
            nc.vector.scalar_tensor_tensor(out=diff, in0=iok, scalar=1.0,
                                           in1=_bcast_inner(uf, K),
                                           op0=_ALU.mult, op1=_ALU.subtract)
            # w = 1.0*(diff==0) + sum_m DECAYS[m]*(|diff|==m)
            w = small.tile([P, TILES, K], mybir.dt.float32)
            nc.vector.tensor_scalar(out=w, in0=diff, scalar1=0.0, scalar2=None,
                                    op0=_ALU.is_equal)
            tmp = small.tile([P, TILES, K], mybir.dt.float32)
            for m in (1, 2, 3):
                for s in (-m, m):
                    nc.vector.tensor_scalar(out=tmp, in0=diff, scalar1=float(s),
                                            scalar2=None, op0=_ALU.is_equal)
                    nc.vector.scalar_tensor_tensor(out=w, in0=tmp, scalar=DECAYS[m],
                                                   in1=w, op0=_ALU.mult, op1=_ALU.add)
            wsum = small.tile([P, TILES], mybir.dt.float32)
            nc.vector.reduce_sum(out=wsum, in_=w, axis=mybir.AxisListType.X)

            # gather-side term, computed early (off the tail)
            wg = small.tile([P, TILES, K], mybir.dt.float32)
            gsum = small.tile([P, TILES], mybir.dt.float32)
            nc.vector.tensor_mul(out=wg, in0=w, in1=gath)
            nc.vector.reduce_sum(out=gsum, in_=wg, axis=mybir.AxisListType.X)

            # dense stream: per-token sum(exp), split ACT-accum / DVE-reduce.
            # Every ACTIVATE carries accum_out (the HW accumulator register
            # is stateful across ops; pairing each op with a read is the
            # proven-correct pattern); batched groups dump theirs into a
            # junk column.
            sums = small.tile([P, TILES], mybir.dt.float32)
            junk = small.tile([P, 1], mybir.dt.float32)
            lse = small.tile([P, TILES], mybir.dt.float32)
            for jg in range(ngroups):
                pt = pred_pool.tile([P, G, C], mybir.dt.float32)
                nc.sync.dma_start(out=pt, in_=pred_g[jg])
                if jg in acc_set:
                    for g in range(G):
                        j = jg * G + g
                        et = exp_pool.tile([P, C], mybir.dt.float32, tag="acc")
                        nc.scalar.activation(out=et, in_=pt[:, g, :], func=_ACT.Exp,
                                             accum_out=sums[:, j:j + 1])
                else:
                    et = exp_pool.tile([P, G, C], mybir.dt.float32, tag="dve")
                    nc.scalar.activation(out=et, in_=pt, func=_ACT.Exp,
                                         accum_out=junk)
                    nc.vector.reduce_sum(out=sums[:, jg * G:(jg + 1) * G], in_=et,
                                         axis=mybir.AxisListType.X)
                if ln_split and (jg + 1) * G == ln_split:
                    # bulk Ln mid-stream: hides the ln-table load under the
                    # stream (the exp table reloads before the next group,
                    # also hidden)
                    nc.scalar.activation(out=lse[:, :ln_split],
                                         in_=sums[:, :ln_split], func=_ACT.Ln)

            if ln_split:
                nc.scalar.activation(out=lse[:, ln_split:], in_=sums[:, ln_split:],
                                     func=_ACT.Ln)
            else:
                nc.scalar.activation(out=lse, in_=sums, func=_ACT.Ln)
            loss = small.tile([P, TILES], mybir.dt.float32)
            nc.vector.tensor_mul(out=loss, in0=wsum, in1=lse)
            nc.vector.tensor_sub(out=loss, in0=loss, in1=gsum)
            part = small.tile([P, 1], mybir.dt.float32)
            nc.vector.reduce_sum(out=part, in_=loss, axis=mybir.AxisListType.X)
            nc.sync.dma_start(out=out.ap(), in_=part)
    nc.compile()
    return nc


def _get_nc():
    global _NC
    if _NC is None:
        _NC = _build()
    return _NC


def _shard_inputs(pred, target):
    bpc = B // CORES
    in_maps = []
    for c in range(CORES):
        in_maps.append({
            "pred": np.ascontiguousarray(
                pred[c * bpc:(c + 1) * bpc].reshape(SHARD, C), dtype=np.float32),
            "target": np.ascontiguousarray(
                target[c * bpc:(c + 1) * bpc].reshape(SHARD), dtype=np.int32),
        })
    return in_maps


def _run(pred, target, **kwargs):
    nc = _get_nc()
    return bass_utils.run_bass_kernel_spmd(
        nc, _shard_inputs(pred, target), core_ids=list(range(CORES)), **kwargs)


def kernel(pred, target):
    res = _run(pred, target)
    total = sum(float(r["partial"].astype(np.float64).sum()) for r in res.results)
    return np.asarray(total / (B * T), dtype=np.float32)


# revision 8
# speedup vs baseline: 1.0460x; 1.0460x over previous
"""Trainium2 Bass kernel: cross-entropy with Gaussian-smoothed labels.

loss = mean over tokens of  [ Wsum(t) * logsumexp(pred_row) - sum_k w_k * pred[start+k] ]

where the smoothed one-hot reduces exactly to a 7-tap window:
  start = clip(t-3, 0, C-7), u = t-start, w_k = f(k-u)
  f(0)=1.0, f(+-m)=exp(-2^m/4) for m in 1..3, else 0.

Sharding: pure data-parallel over the batch axis, 4 batches (8192 tokens)
per core across 8 cores. Per core:
  - stream pred [8192, 722] f32 through SBUF in [128, G*722] tiles.
    Per-token sum(exp) is load-balanced between the ACT engine
    (exp with HW accumulator, one op per token column) and DVE
    (one batched Exp + reduce_sum per group) so neither engine
    outruns the DMA stream; Ln is split so the ln-table load hides
    under the stream.
  - 64 indirect DMAs gather the 7-wide windows (one offset per
    partition per op — the only offset layout the SWDGE firmware
    honors; multi-offset-per-partition reads garbage on HW even
    though CoreSim accepts it).
  - weights built once on-chip from iota/compares ([128, 64, 7]);
    the whole gather-side term is computed early, off the tail.
  - per-core partial sums [128, 1] DMA'd out; host sums 8x128 and divides.
"""
import math

import numpy as np

import concourse.bass as bass
import concourse.bacc as bacc
import concourse.tile as tile
from concourse import mybir
from concourse import bass_utils

B, T, C = 32, 2048, 722
CORES = 8
SHARD = B * T // CORES          # 8192 tokens per core
P = 128
TILES = SHARD // P              # 64
K = 7
START_MAX = C - K               # 715
DECAYS = [math.exp(-(2.0 ** d) / 4.0) for d in range(4)]

_ALU = mybir.AluOpType
_ACT = mybir.ActivationFunctionType

_NC = None


def _bcast_inner(ap, n):
    """Append a step-0 broadcast dim of length n to an AP."""
    return bass.AP(tensor=ap.tensor, offset=ap.offset, ap=[*ap.ap, [0, n]])


def _build(G=4, acc_set=(3, 6, 9, 12, 15), ln_split=0, pred_bufs=4, exp_bufs=3):
    ngroups = TILES // G
    acc_set = set(acc_set)
    nc = bacc.Bacc("TRN2", target_bir_lowering=False, debug=False,
                   enable_asserts=True, num_devices=CORES)
    pred = nc.dram_tensor("pred", [SHARD, C], mybir.dt.float32, kind="ExternalInput")
    target = nc.dram_tensor("target", [SHARD], mybir.dt.int32, kind="ExternalInput")
    out = nc.dram_tensor("partial", [P, 1], mybir.dt.float32, kind="ExternalOutput")

    pred_flat = pred.ap().rearrange("a b -> (a b)").rearrange("(n one) -> n one", one=1)
    # token index = p*TILES + jg*G + g  (each partition owns a contiguous slab)
    pred_g = pred.ap().rearrange("(p j g) c -> j p g c", p=P, g=G)

    with tile.TileContext(nc) as tc:
        with (tc.tile_pool(name="pred", bufs=pred_bufs) as pred_pool,
              tc.tile_pool(name="exp", bufs=exp_bufs) as exp_pool,
              tc.tile_pool(name="small", bufs=1) as small):
            # targets: tgt_sb[p, j] = target[j*128 + p]; scalar ring keeps
            # the sync ring exclusively for the pred stream.
            tgt_sb = small.tile([P, TILES], mybir.dt.int32)
            nc.scalar.dma_start(out=tgt_sb,
                                in_=target.ap().rearrange("(p j) -> p j", p=P))

            # flat element offsets of each token's window start
            row = small.tile([P, TILES], mybir.dt.int32)
            nc.gpsimd.iota(row, pattern=[[1, TILES]], base=0, channel_multiplier=TILES)
            start_i = small.tile([P, TILES], mybir.dt.int32)
            nc.vector.tensor_scalar(out=start_i, in0=tgt_sb, scalar1=3, scalar2=0,
                                    op0=_ALU.subtract, op1=_ALU.max)
            nc.vector.tensor_scalar_min(out=start_i, in0=start_i, scalar1=START_MAX)
            offs = small.tile([P, TILES], mybir.dt.int32)
            nc.vector.tensor_scalar_mul(out=offs, in0=row, scalar1=C)
            nc.vector.tensor_add(out=offs, in0=offs, in1=start_i)

            # windowed gathers: one indirect DMA per token-tile, one offset
            # per partition (the only HW-correct layout)
            gath = small.tile([P, TILES, K], mybir.dt.float32)
            for j in range(TILES):
                nc.gpsimd.indirect_dma_start(
                    out=gath[:, j, :],
                    out_offset=None,
                    in_=pred_flat,
                    in_offset=bass.IndirectOffsetOnAxis(ap=offs[:, j:j + 1], axis=0),
                )

            # u = t - start in f32 (0..6); diff[p,j,k] = k - u[p,j]
            tf = small.tile([P, TILES], mybir.dt.float32)
            nc.vector.tensor_copy(out=tf, in_=tgt_sb)
            sf = small.tile([P, TILES], mybir.dt.float32)
            nc.vector.tensor_copy(out=sf, in_=start_i)
            uf = small.tile([P, TILES], mybir.dt.float32)
            nc.vector.tensor_sub(out=uf, in0=tf, in1=sf)

            iok = small.tile([P, TILES, K], mybir.dt.float32)
            nc.gpsimd.iota(iok, pattern=[[0, TILES], [1, K]], base=0,
                           channel_multiplier=0, allow_small_or_imprecise_dtypes=True)
            diff = small.tile([P, TILES, K], mybir.dt.float32)
            nc.vector.scalar_tensor_tensor(out=diff, in0=iok, scalar=1.0,
                                           in1=_bcast_inner(uf, K),
                                           op0=_ALU.mult, op1=_ALU.subtract)
            # w = 1.0*(diff==0) + sum_m DECAYS[m]*(|diff|==m)
            w = small.tile([P, TILES, K], mybir.dt.float32)
            nc.vector.tensor_scalar(out=w, in0=diff, scalar1=0.0, scalar2=None,
                                    op0=_ALU.is_equal)
            tmp = small.tile([P, TILES, K], mybir.dt.float32)
            for m in (1, 2, 3):
                for s in (-m, m):
                    nc.vector.tensor_scalar(out=tmp, in0=diff, scalar1=float(s),
                                            scalar2=None, op0=_ALU.is_equal)
                    nc.vector.scalar_tensor_tensor(out=w, in0=tmp, scalar=DECAYS[m],
                                                   in1=w, op0=_ALU.mult, op1=_ALU.add)
            wsum = small.tile([P, TILES], mybir.dt.float32)
            nc.vector.reduce_sum(out=wsum, in_=w, axis=mybir.AxisListType.X)

            # dense stream: per-token sum(exp), split ACT-accum / DVE-reduce.
            # Every ACTIVATE carries accum_out (the HW accumulator register
            # is stateful across ops; pairing each op with a read is the
            # proven-correct pattern); batched groups dump theirs into a
            # junk column.
            sums = small.tile([P, TILES], mybir.dt.float32)
            junk = small.tile([P, 1], mybir.dt.float32)
            lse = small.tile([P, TILES], mybir.dt.float32)
            for jg in range(ngroups):
                pt = pred_pool.tile([P, G, C], mybir.dt.float32)
                nc.sync.dma_start(out=pt, in_=pred_g[jg])
                if jg in acc_set:
                    for g in range(G):
                        j = jg * G + g
                        et = exp_pool.tile([P, C], mybir.dt.float32, tag="acc")
                        nc.scalar.activation(out=et, in_=pt[:, g, :], func=_ACT.Exp,
                                             accum_out=sums[:, j:j + 1])
                else:
                    et = exp_pool.tile([P, G, C], mybir.dt.float32, tag="dve")
                    nc.scalar.activation(out=et, in_=pt, func=_ACT.Exp,
                                         accum_out=junk)
                    nc.vector.reduce_sum(out=sums[:, jg * G:(jg + 1) * G], in_=et,
                                         axis=mybir.AxisListType.X)
                if ln_split and (jg + 1) * G == ln_split:
                    # bulk Ln mid-stream: hides the ln-table load under the
                    # stream (the exp table reloads before the next group,
                    # also hidden)
                    nc.scalar.activation(out=lse[:, :ln_split],
                                         in_=sums[:, :ln_split], func=_ACT.Ln)

            if ln_split:
                nc.scalar.activation(out=lse[:, ln_split:], in_=sums[:, ln_split:],
                                     func=_ACT.Ln)
            else:
                nc.scalar.activation(out=lse, in_=sums, func=_ACT.Ln)
            # gather-side term: must stay AFTER the stream in program order —
            # DVE executes in order, and an early op waiting on all 64
            # gathers (~70us of Q7) head-of-line blocks every stream reduce
            # behind it, stalling the whole pipeline.
            wg = small.tile([P, TILES, K], mybir.dt.float32)
            gsum = small.tile([P, TILES], mybir.dt.float32)
            nc.vector.tensor_mul(out=wg, in0=w, in1=gath)
            nc.vector.reduce_sum(out=gsum, in_=wg, axis=mybir.AxisListType.X)
            loss = small.tile([P, TILES], mybir.dt.float32)
            nc.vector.tensor_mul(out=loss, in0=wsum, in1=lse)
            nc.vector.tensor_sub(out=loss, in0=loss, in1=gsum)
            part = small.tile([P, 1], mybir.dt.float32)
            nc.vector.reduce_sum(out=part, in_=loss, axis=mybir.AxisListType.X)
            nc.sync.dma_start(out=out.ap(), in_=part)
    nc.compile()
    return nc


def _get_nc():
    global _NC
    if _NC is None:
        _NC = _build()
    return _NC


def _shard_inputs(pred, target):
    bpc = B // CORES
    in_maps = []
    for c in range(CORES):
        in_maps.append({
            "pred": np.ascontiguousarray(
                pred[c * bpc:(c + 1) * bpc].reshape(SHARD, C), dtype=np.float32),
            "target": np.ascontiguousarray(
                target[c * bpc:(c + 1) * bpc].reshape(SHARD), dtype=np.int32),
        })
    return in_maps


def _run(pred, target, **kwargs):
    nc = _get_nc()
    return bass_utils.run_bass_kernel_spmd(
        nc, _shard_inputs(pred, target), core_ids=list(range(CORES)), **kwargs)


def kernel(pred, target):
    res = _run(pred, target)
    total = sum(float(r["partial"].astype(np.float64).sum()) for r in res.results)
    return np.asarray(total / (B * T), dtype=np.float32)
